# revision 61
# baseline (speedup 1.0000x reference)
"""Fused sparse-attention kernel for Trainium2 (8 NeuronCores, data-parallel over batch).

Computation (per batch element b):
    X[s,k]  = enc[b] @ W_enc + dec_proj[b,k] + cov[b,s]*Wcovsum[k] + bias[k]
    T       = tanh(X)
    att[s]  = T @ v_w                      (+ v_b, which cancels in softmax)
    w       = softmax(att masked to s < len[b])
    new_cov = cov + w

Sharding: batch B=32 is split 4-per-core across 8 cores; all weights replicated
(per the data-parallel sharding hint).

Length-specialized fp8 DoubleRowSwInterleave pipeline (126.7us baseline ->
46.4us, TimelineSim per-core):

  Host planning: batches are sorted by text_length and snake-assigned so all
  8 cores carry identical per-slot live-tile counts njts (even, ascending,
  e.g. (4, 8, 14, 16)); the Bass program is compiled specialized to njts
  (cached per pattern) and simply never emits work for s-tiles that the
  length mask would zero anyway. Outputs for dead tiles come from a memset
  att=0 + the mask in the softmax tail, so results are bit-identical to the
  unspecialized program.

  Per batch slot (only live j-tiles, j < njts[b]):
  1. SWDGE cast-DMA: enc[b] fp32 DRAM -> fp8e4m3 DRAM scratch [s,h]
     (contiguous 64KB descriptors), in bh-halves matching transpose rounds.
  2. Xbar DMA-transposes of the scratch viewed as uint16 h-PAIRS: one
     [live_s x 128pair] transpose per (bh, pc) lands DIRECTLY in SBUF
     et2[pc] as s-major interleaved fp8 pairs (h=2hh, h=2hh+1) -- the
     contiguous A/B-interleaved stationary layout DoubleRowSwInterleave
     consumes. No PSUM bounce, no engine copies; the 16x128 xbar tiles
     cost 14ns each (~9.4us total on the DMA engines, which sit at ~40%).
     Freed PSUM banks fund a 3-deep mains pipeline (ppm bufs=3).
  3. PE per s-tile psum group (two groups share a 2-bank [128,1024] tile):
     K=2 bf16 rank-1 (ones,cov~) x (16*(dec_proj+b), 16*Wcovsum) + 2 fp8
     DoubleRowSwInterleave matmuls (K=256 each, 0.5 cyc/row) with moving
     W2[pc][hh,t,k] = 16*W_enc[pc*256+2hh+t, k]. SwInterleave reads
     stationary columns reversed, so out partition p within an s-tile is
     s = 128j + 127 - p; all downstream per-partition constants (iota, cov
     tiles, rank-1 cov rows) and the host unshard are flipped to match.
     The x16 W scaling keeps W_enc (std 0.02) out of fp8e4m3's denormal
     range; tanh's scale=1/16 undoes it.
  4. ACT: one tanh per j-PAIR ([128,1024] psum -> bf16 T tile).
  5. DVE: fused T*v multiply + free-dim reduce -> att column [128,1].
  6. Masked softmax tail in [s_lo=128, s_hi=16] layout: exp on ACT, iota<len
     mask fused with the exp multiply on DVE, fp32 partition-sum + 1/sum
     broadcast via two small PE matmuls (the [1,16] fp32 matmul costs ~27ns,
     cheaper than a bf16 bounce; max-subtraction skipped: |logits| <=
     ||v||_1 ~ 8, and v_b cancels in softmax); wts and ncov both derive
     from mexp x inv as independent siblings. Each batch's tail is emitted
     in 4 chunks spread across the next batch's main loop so its serial
     chain hides in the in-order engine queues; constants load on the idle
     SP queue and outputs DMA out on SP as well.
"""

import numpy as np
import ml_dtypes

B, S, H, E = 32, 2048, 512, 512
NCORES = 8
BPC = B // NCORES           # batches per core
SLO, SHI = 128, S // 128    # att tile layout: s = 128*j + (127-p)  ->  [p, j]
NPC = 2                     # pair-chunks of 128 uint16 pairs (256 h) each
BF16 = ml_dtypes.bfloat16
F8E4 = ml_dtypes.float8_e4m3
WSCALE = 16.0

_CACHE = {}


def _build_nc(njts):
    import concourse.mybir as mybir
    import concourse.tile as tile
    from concourse import bacc
    from contextlib import ExitStack

    dt = mybir.dt
    F32, BF, F8, F16 = dt.float32, dt.bfloat16, dt.float8e4, dt.float16

    nc = bacc.Bacc("TRN2", target_bir_lowering=False, debug=False,
                   enable_asserts=False, num_devices=NCORES)

    # ---- DRAM I/O (per-core shapes) ----
    enc_f32 = nc.dram_tensor("enc_f32", [BPC, S, H], F32, kind="ExternalInput").ap()
    # fp8 moving weights: per pc, per t: 512 cols of W_enc*16
    wdr = nc.dram_tensor("wdr", [128, NPC * 2 * H], F8, kind="ExternalInput").ap()
    idn = nc.dram_tensor("idn", [128, 128], F16, kind="ExternalInput").ap()
    # f32 blob: [iota (SHI) | lens (BPC) | cov_t (BPC*SHI)]   (s-flipped layout)
    fblob = nc.dram_tensor("fblob", [SLO, SHI + BPC + BPC * SHI + 1], F32,
                           kind="ExternalInput").ap()
    r1lhs = nc.dram_tensor("r1lhs", [2, BPC * S], BF, kind="ExternalInput").ap()
    r1rhs = nc.dram_tensor("r1rhs", [2, BPC * H], BF, kind="ExternalInput").ap()
    # bf16 row consts: [vbc (H) | ones col (1)] per partition
    vbc = nc.dram_tensor("vbc", [128, H + 1], BF, kind="ExternalInput").ap()
    brow = nc.dram_tensor("brow", [1, 128], F32, kind="ExternalInput").ap()
    att_out = nc.dram_tensor("att_out", [BPC, SLO, SHI], F32, kind="ExternalOutput").ap()
    cov_out = nc.dram_tensor("cov_out", [BPC, SLO, SHI], F32, kind="ExternalOutput").ap()

    AF = mybir.ActivationFunctionType
    OP = mybir.AluOpType
    PM = mybir.MatmulPerfMode

    with tile.TileContext(nc) as tc, ExitStack() as ctx:
        consts = ctx.enter_context(tc.tile_pool(name="consts", bufs=1))
        encp = ctx.enter_context(tc.tile_pool(name="encp", bufs=2))
        etp = ctx.enter_context(tc.tile_pool(name="etp", bufs=2))
        tpool = ctx.enter_context(tc.tile_pool(name="tpool", bufs=4))
        spool = ctx.enter_context(tc.tile_pool(name="spool", bufs=2))
        small = ctx.enter_context(tc.tile_pool(name="small", bufs=2))
        attp = ctx.enter_context(tc.tile_pool(name="attp", bufs=1))
        dramp = ctx.enter_context(tc.tile_pool(name="dramp", bufs=2, space="DRAM"))
        ppm = ctx.enter_context(tc.tile_pool(name="ppm", bufs=3, space="PSUM"))
        pps = ctx.enter_context(tc.tile_pool(name="pps", bufs=2, space="PSUM"))

        # ---- one-time constant loads (emitted first on the Pool queue) ----
        idn_sb = consts.tile([128, 128], F16, tag="idn")
        nc.sync.dma_start(idn_sb[:], idn[:])
        wdr_sb = consts.tile([128, NPC * 2 * H], F8, tag="wdr")
        nc.sync.dma_start(wdr_sb[:], wdr[:])
        r1lhs_sb = consts.tile([2, BPC * S], BF, tag="r1lhs")
        nc.sync.dma_start(r1lhs_sb[:], r1lhs[:])
        r1rhs_sb = consts.tile([2, BPC * H], BF, tag="r1rhs")
        nc.sync.dma_start(r1rhs_sb[:], r1rhs[:])
        fb_sb = consts.tile([SLO, SHI + BPC + BPC * SHI + 1], F32, tag="fblob")
        vbc_sb = consts.tile([128, H + 1], BF, tag="vbc")
        brow_sb = consts.tile([1, 128], F32, tag="brow")

        iota_sb = fb_sb[:, 0:SHI]
        lens_sb = fb_sb[:, SHI:SHI + BPC]
        covt_sb = fb_sb[:, SHI + BPC:SHI + BPC + BPC * SHI]
        ones_cf_sb = fb_sb[:, SHI + BPC + BPC * SHI:]      # [128,1] f32 ones
        ones_c_sb = vbc_sb[:, H:H + 1]                     # [128,1] bf16 ones
        ones_r_sb = brow_sb                                # [1,128] f32 ones

        def wdr_ap(pc):  # [128, 2, H] fp8 moving pair weights
            return wdr_sb[:, pc * 2 * H:(pc + 1) * 2 * H].rearrange(
                "p (t k) -> p t k", t=2)

        # ---- per-batch cast: fp32 DRAM -> fp8 DRAM scratch [s,h], bh-halves ----
        def load_batch(b):
            e8d = dramp.tile([S, H], F8, tag="e8d")
            for bh in range(-(-njts[b] // 8)):
                rlo, rhi = bh * 1024, min(njts[b], bh * 8 + 8) * 128
                nc.gpsimd.dma_start(e8d[rlo:rhi, :], enc_f32[b, rlo:rhi, :])
            return e8d

        # xbar DMA-transposes: fp8 DRAM scratch viewed as uint16 h-pairs,
        # one transpose per (bh, pc) lands directly in the pair-interleaved
        # SBUF et2 layout that DoubleRowSwInterleave consumes (no PSUM
        # bounce, no engine copies). Returned as closures so the caller can
        # spread them through the previous batch's main loop.
        def make_rounds(e8d, b):
            njt = njts[b]
            e8u = e8d[:].bitcast(mybir.dt.uint16)    # [S, 256] pair view
            et2 = [etp.tile([128, 2 * S], F8, tag=f"et2_{pc}", name=f"et2_{pc}")
                   for pc in range(NPC)]
            nbh = -(-njt // 8)
            rounds = []
            for bh in range(nbh):
                for pc in range(NPC):
                    def emit(bh=bh, pc=pc):
                        jlo, jhi = bh * 8, min(njt, bh * 8 + 8)
                        nj = jhi - jlo
                        dst = et2[pc][:, bh * S:bh * S + nj * 256]
                        nc.sync.dma_start(
                            dst.bitcast(mybir.dt.uint16),
                            e8u[jlo * 128:jhi * 128, pc * 128:(pc + 1) * 128],
                            transpose=True)
                    rounds.append(emit)
            return et2, rounds

        # ---- masked softmax tail, emitted in 4 chunks so the serial chain
        # interleaves with the next batch's main loop on the in-order queues ----
        tail_state = {}

        def tail_chunk(b, att_t, phase):
            st = tail_state.setdefault(b, {})
            if phase == 0:
                st["expt"] = small.tile([SLO, SHI], F32, tag="expt", name="expt")
                nc.scalar.activation(st["expt"][:], att_t[:], AF.Exp)
                st["mexp"] = small.tile([SLO, SHI], F32, tag="mexp", name="mexp")
                nc.vector.scalar_tensor_tensor(
                    out=st["mexp"][:], in0=iota_sb, scalar=lens_sb[:, b:b + 1],
                    in1=st["expt"][:], op0=OP.is_lt, op1=OP.mult,
                )
                st["pst"] = pps.tile([128, 32], F32, tag="smax", name="smax")
                nc.tensor.matmul(st["pst"][0:1, 0:SHI], ones_cf_sb,
                                 st["mexp"][:],
                                 start=True, stop=True, skip_group_check=True)
            elif phase == 1:
                ssum = small.tile([1, 1], F32, tag="ssum")
                nc.vector.reduce_sum(ssum[:], st["pst"][0:1, 0:SHI],
                                     axis=mybir.AxisListType.X)
                sinv = small.tile([1, 1], F32, tag="sinv")
                nc.vector.reciprocal(sinv[:], ssum[:])
                nc.tensor.matmul(st["pst"][:, 16:17], ones_r_sb, sinv[:],
                                 start=True, stop=True, skip_group_check=True)
            elif phase == 2:
                st["wts"] = small.tile([SLO, SHI], F32, tag="wts", name="wts")
                nc.vector.tensor_scalar(st["wts"][:], st["mexp"][:],
                                        st["pst"][:, 16:17], None, OP.mult)
                nc.sync.dma_start(att_out[b], st["wts"][:])
            else:
                ncov = small.tile([SLO, SHI], F32, tag="ncov")
                nc.vector.scalar_tensor_tensor(
                    out=ncov[:], in0=st["mexp"][:], scalar=st["pst"][:, 16:17],
                    in1=covt_sb[:, b * SHI:(b + 1) * SHI],
                    op0=OP.mult, op1=OP.add,
                )
                nc.sync.dma_start(cov_out[b], ncov[:])
                del tail_state[b]

        def mains_pair(b, jp, att_t, et2):
            ps = ppm.tile([128, 2 * H], F32, tag="x")
            for g in range(2):
                j = 2 * jp + g
                psg = ps[:, g * H:(g + 1) * H]
                nc.tensor.matmul(
                    psg,
                    r1lhs_sb[:, b * S + j * 128: b * S + (j + 1) * 128],
                    r1rhs_sb[:, b * H:(b + 1) * H],
                    start=True, stop=False, skip_group_check=True,
                )
                for pc in range(NPC):
                    nc.tensor.matmul(
                        psg,
                        et2[pc][:, j * 256:(j + 1) * 256],
                        wdr_ap(pc),
                        start=False, stop=(pc == NPC - 1),
                        perf_mode=PM.DoubleRowSwInterleave,
                        skip_group_check=True,
                    )
            t_t = tpool.tile([128, 2 * H], BF, tag="t")
            nc.scalar.activation(t_t[:], ps[:], AF.Tanh, scale=1.0 / WSCALE)
            for g in range(2):
                j = 2 * jp + g
                scr = spool.tile([128, H], BF, tag=f"scr{g}")
                nc.vector.scalar_tensor_tensor(
                    out=scr[:], in0=t_t[:, g * H:(g + 1) * H], scalar=1.0,
                    in1=vbc_sb[:, 0:H],
                    op0=OP.mult, op1=OP.mult,
                    accum_out=att_t[:, j:j + 1],
                )

        # ---- stage schedule: merge latency-bound small batches so their
        # independent pipeline chains overlap ----
        stages = []
        b = 0
        while b < BPC:
            stages.append([b])
            b += 1

        pre, et2s, att_ts = {}, {}, {}
        for b in stages[0]:
            pre[b] = load_batch(b)
        # late-needed consts ride the Pool/SWDGE queue behind the first casts
        nc.gpsimd.dma_start(vbc_sb[:], vbc[:])
        nc.gpsimd.dma_start(fb_sb[:], fblob[:])
        nc.gpsimd.dma_start(brow_sb[:], brow[:])
        for b in stages[0]:
            et2s[b], r0 = make_rounds(pre.pop(b), b)
            for r in r0:
                r()

        prev_tails = []    # (b, att_t) awaiting tail emission
        for si, stage in enumerate(stages):
            nxt = stages[si + 1] if si + 1 < len(stages) else []
            rounds_next = []
            for nb in nxt:
                pre[nb] = load_batch(nb)
            for nb in nxt:
                et2s[nb], rs = make_rounds(pre.pop(nb), nb)
                rounds_next.extend(rs)

            # round-robin the stage's pair groups
            seq = []
            for b in stage:
                att_ts[b] = attp.tile([SLO, SHI], F32, tag=f"att{b}",
                                      name=f"att{b}")
                if njts[b] < SHI:
                    nc.gpsimd.memset(att_ts[b][:, njts[b]:], 0.0)
            for jp in range(max(njts[b] // 2 for b in stage)):
                for b in stage:
                    if jp < njts[b] // 2:
                        seq.append((b, jp))

            nchunks = 4 * len(prev_tails)
            chunks_done = 0
            npos = len(seq)
            for pos, (b, jp) in enumerate(seq):
                mains_pair(b, jp, att_ts[b], et2s[b])
                # pending softmax tails from the previous stage
                target = (pos + 1) * nchunks // npos
                while chunks_done < min(target, nchunks):
                    tb, tatt = prev_tails[chunks_done // 4]
                    tail_chunk(tb, tatt, chunks_done % 4)
                    chunks_done += 1
                # next stage's transpose rounds, late in this stage's loop
                k = pos - max(1, npos - len(rounds_next))
                if 0 <= k < len(rounds_next):
                    rounds_next[k]()
                    rounds_next[k] = None
            while chunks_done < nchunks:
                tb, tatt = prev_tails[chunks_done // 4]
                tail_chunk(tb, tatt, chunks_done % 4)
                chunks_done += 1
            for r in rounds_next:
                if r is not None:
                    r()

            prev_tails = [(b, att_ts[b]) for b in stage]

        for tb, tatt in prev_tails:
            for phase in range(4):
                tail_chunk(tb, tatt, phase)

    nc.compile()
    return nc


def _get_nc(njts=None):
    if njts is None:
        return _CACHE["last"]
    if njts not in _CACHE:
        _CACHE[njts] = _build_nc(njts)
    _CACHE["last"] = _CACHE[njts]
    return _CACHE[njts]


def _plan(text_lengths):
    """Sort batches by length, snake-assign to cores so every core sees the
    same per-slot live-tile count (SPMD), shortest slot first."""
    lens_i = np.clip(np.asarray(text_lengths).astype(np.int64), 1, S)
    order = np.argsort(-lens_i, kind="stable")
    groups = [order[8 * g:8 * g + 8] for g in range(BPC)]   # g=0 longest
    groups = groups[::-1]                                   # ascending njt
    njts = tuple(int(2 * -(-int(lens_i[g].max()) // 256)) for g in groups)
    # bidx[core][slot] = original batch index
    bidx = [[int(groups[s][c]) for s in range(BPC)] for c in range(NCORES)]
    return njts, bidx


def _prep_in_maps(dec_input, enc_output, text_lengths, coverage_vector, W, b,
                  v_w, bidx):
    enc = np.ascontiguousarray(np.asarray(enc_output, dtype=np.float32))
    dec = np.asarray(dec_input, dtype=np.float32).reshape(B, E)
    cov = np.asarray(coverage_vector, dtype=np.float32)
    W = np.asarray(W, dtype=np.float32)
    b = np.asarray(b, dtype=np.float32)
    v_w = np.asarray(v_w, dtype=np.float32)
    lens_f = np.asarray(text_lengths).astype(np.float32)

    wenc16 = (W[:H] * WSCALE).astype(F8E4)      # [h, k] fp8, x16
    wcovsum = W[H + E:].sum(axis=0, dtype=np.float32)
    dec_proj = dec @ W[H:H + E]                 # (B, H) fp32 on host
    vbc = np.empty((128, H + 1), BF16)
    vbc[:, :H] = v_w.astype(BF16)[None, :]
    vbc[:, H] = BF16(1.0)
    # SwInterleave reverses stationary columns: partition p <-> s = 128j+127-p
    iota = ((127.0 - np.arange(SLO, dtype=np.float32))[:, None]
            + 128.0 * np.arange(SHI, dtype=np.float32)[None, :])
    brow = np.ones((1, 128), np.float32)
    # cov in [p, j] layout with the s flip inside each 128-block
    cov_pj = cov.reshape(B, SHI, SLO)[:, :, ::-1].transpose(0, 2, 1)  # [B,128,SHI]

    wdr = np.zeros((128, NPC * 2 * H), F8E4)
    hh = np.arange(128)
    for pc in range(NPC):
        for t in range(2):
            rows = wenc16[pc * 256 + 2 * hh + t]            # [128, H]
            wdr[:, (pc * 2 + t) * H:(pc * 2 + t + 1) * H] = rows
    idn = np.eye(128, dtype=np.float16)

    in_maps = []
    for core in range(NCORES):
        sl = bidx[core]

        fblob = np.empty((SLO, SHI + BPC + BPC * SHI + 1), np.float32)
        fblob[:, 0:SHI] = iota
        fblob[:, SHI:SHI + BPC] = lens_f[sl][None, :]
        fblob[:, SHI + BPC:SHI + BPC + BPC * SHI] = \
            cov_pj[sl].transpose(1, 0, 2).reshape(SLO, BPC * SHI)
        fblob[:, -1] = 1.0

        r1l = np.empty((2, BPC * S), BF16)
        r1l[0] = BF16(1.0)
        # r1 columns map straight to out partitions: use the flipped layout
        r1l[1] = (cov_pj[sl].astype(BF16).transpose(0, 2, 1).reshape(-1))

        r1r = np.empty((2, BPC * H), np.float32)
        r1r[0] = (WSCALE * (dec_proj[sl] + b[None, :])).reshape(-1)
        r1r[1] = np.broadcast_to(WSCALE * wcovsum, (BPC, H)).reshape(-1)

        in_maps.append({
            "enc_f32": enc[sl],
            "wdr": wdr,
            "idn": idn,
            "fblob": fblob,
            "r1lhs": r1l,
            "r1rhs": r1r.astype(BF16),
            "vbc": vbc,
            "brow": brow,
        })
    return in_maps


def kernel(dec_input, enc_output, text_lengths, coverage_vector, W, b, v_w, v_b):
    from concourse.bass_utils import run_bass_kernel_spmd

    njts, bidx = _plan(text_lengths)
    nc = _get_nc(njts)
    in_maps = _prep_in_maps(dec_input, enc_output, text_lengths,
                            coverage_vector, W, b, v_w, bidx)
    res = run_bass_kernel_spmd(nc, in_maps, core_ids=list(range(NCORES)))

    att = np.empty((B, S), np.float32)
    ncov = np.empty((B, S), np.float32)
    for core in range(NCORES):
        r = res.results[core]
        # undo the per-128-block s flip: out partition p is s = 128j + 127 - p
        att[bidx[core]] = r["att_out"][:, ::-1, :].transpose(0, 2, 1).reshape(BPC, S)
        ncov[bidx[core]] = r["cov_out"][:, ::-1, :].transpose(0, 2, 1).reshape(BPC, S)
    return att, ncov


# revision 62
# speedup vs baseline: 1.0269x; 1.0269x over previous
"""Fused sparse-attention kernel for Trainium2 (8 NeuronCores, data-parallel over batch).

Computation (per batch element b):
    X[s,k]  = enc[b] @ W_enc + dec_proj[b,k] + cov[b,s]*Wcovsum[k] + bias[k]
    T       = tanh(X)
    att[s]  = T @ v_w                      (+ v_b, which cancels in softmax)
    w       = softmax(att masked to s < len[b])
    new_cov = cov + w

Sharding: batch B=32 is split 4-per-core across 8 cores; all weights replicated
(per the data-parallel sharding hint).

Length-specialized fp8 DoubleRowSwInterleave pipeline (126.7us baseline ->
46.4us, TimelineSim per-core):

  Host planning: batches are sorted by text_length and snake-assigned so all
  8 cores carry identical per-slot live-tile counts njts (even, ascending,
  e.g. (4, 8, 14, 16)); the Bass program is compiled specialized to njts
  (cached per pattern) and simply never emits work for s-tiles that the
  length mask would zero anyway. Outputs for dead tiles come from a memset
  att=0 + the mask in the softmax tail, so results are bit-identical to the
  unspecialized program.

  Per batch slot (only live j-tiles, j < njts[b]):
  1. SWDGE cast-DMA: enc[b] fp32 DRAM -> fp8e4m3 DRAM scratch [s,h]
     (contiguous 64KB descriptors), in bh-halves matching transpose rounds.
  2. Xbar DMA-transposes of the scratch viewed as uint16 h-PAIRS: one
     [live_s x 128pair] transpose per (bh, pc) lands DIRECTLY in SBUF
     et2[pc] as s-major interleaved fp8 pairs (h=2hh, h=2hh+1) -- the
     contiguous A/B-interleaved stationary layout DoubleRowSwInterleave
     consumes. No PSUM bounce, no engine copies; the 16x128 xbar tiles
     cost 14ns each (~9.4us total on the DMA engines, which sit at ~40%).
     Freed PSUM banks fund a 3-deep mains pipeline (ppm bufs=3).
  3. PE per s-tile psum group (two groups share a 2-bank [128,1024] tile):
     K=2 bf16 rank-1 (ones,cov~) x (16*(dec_proj+b), 16*Wcovsum) + 2 fp8
     DoubleRowSwInterleave matmuls (K=256 each, 0.5 cyc/row) with moving
     W2[pc][hh,t,k] = 16*W_enc[pc*256+2hh+t, k]. SwInterleave reads
     stationary columns reversed, so out partition p within an s-tile is
     s = 128j + 127 - p; all downstream per-partition constants (iota, cov
     tiles, rank-1 cov rows) and the host unshard are flipped to match.
     The x16 W scaling keeps W_enc (std 0.02) out of fp8e4m3's denormal
     range; tanh's scale=1/16 undoes it.
  4. ACT: one tanh per j-PAIR ([128,1024] psum -> bf16 T tile).
  5. DVE: fused T*v multiply + free-dim reduce -> att column [128,1].
  6. Masked softmax tail in [s_lo=128, s_hi=16] layout: exp on ACT, iota<len
     mask fused with the exp multiply on DVE, fp32 partition-sum + 1/sum
     broadcast via two small PE matmuls (the [1,16] fp32 matmul costs ~27ns,
     cheaper than a bf16 bounce; max-subtraction skipped: |logits| <=
     ||v||_1 ~ 8, and v_b cancels in softmax); wts and ncov both derive
     from mexp x inv as independent siblings. Each batch's tail is emitted
     in 4 chunks spread across the next batch's main loop so its serial
     chain hides in the in-order engine queues; constants load on the idle
     SP queue and outputs DMA out on SP as well.
"""

import numpy as np
import ml_dtypes

B, S, H, E = 32, 2048, 512, 512
NCORES = 8
BPC = B // NCORES           # batches per core
SLO, SHI = 128, S // 128    # att tile layout: s = 128*j + (127-p)  ->  [p, j]
NPC = 2                     # pair-chunks of 128 uint16 pairs (256 h) each
BF16 = ml_dtypes.bfloat16
F8E4 = ml_dtypes.float8_e4m3
WSCALE = 16.0

_CACHE = {}


def _build_nc(njts):
    import concourse.mybir as mybir
    import concourse.tile as tile
    from concourse import bacc
    from contextlib import ExitStack

    dt = mybir.dt
    F32, BF, F8, F16 = dt.float32, dt.bfloat16, dt.float8e4, dt.float16

    nc = bacc.Bacc("TRN2", target_bir_lowering=False, debug=False,
                   enable_asserts=False, num_devices=NCORES)

    # ---- DRAM I/O (per-core shapes) ----
    enc_f32 = nc.dram_tensor("enc_f32", [BPC, S, H], F32, kind="ExternalInput").ap()
    # fp8 moving weights: per pc, per t: 512 cols of W_enc*16
    wdr = nc.dram_tensor("wdr", [128, NPC * 2 * H], F8, kind="ExternalInput").ap()
    idn = nc.dram_tensor("idn", [128, 128], F16, kind="ExternalInput").ap()
    # f32 blob: [iota (SHI) | lens (BPC) | cov_t (BPC*SHI)]   (s-flipped layout)
    fblob = nc.dram_tensor("fblob", [SLO, SHI + BPC + BPC * SHI + 1], F32,
                           kind="ExternalInput").ap()
    r1lhs = nc.dram_tensor("r1lhs", [2, BPC * S], BF, kind="ExternalInput").ap()
    r1rhs = nc.dram_tensor("r1rhs", [2, BPC * H], BF, kind="ExternalInput").ap()
    # bf16 row consts: [vbc (H) | ones col (1)] per partition
    vbc = nc.dram_tensor("vbc", [128, H + 1], BF, kind="ExternalInput").ap()
    brow = nc.dram_tensor("brow", [1, 128], F32, kind="ExternalInput").ap()
    att_out = nc.dram_tensor("att_out", [BPC, SLO, SHI], F32, kind="ExternalOutput").ap()
    cov_out = nc.dram_tensor("cov_out", [BPC, SLO, SHI], F32, kind="ExternalOutput").ap()

    AF = mybir.ActivationFunctionType
    OP = mybir.AluOpType
    PM = mybir.MatmulPerfMode

    with tile.TileContext(nc) as tc, ExitStack() as ctx:
        consts = ctx.enter_context(tc.tile_pool(name="consts", bufs=1))
        encp = ctx.enter_context(tc.tile_pool(name="encp", bufs=2))
        etp = ctx.enter_context(tc.tile_pool(name="etp", bufs=2))
        tpool = ctx.enter_context(tc.tile_pool(name="tpool", bufs=4))
        spool = ctx.enter_context(tc.tile_pool(name="spool", bufs=2))
        small = ctx.enter_context(tc.tile_pool(name="small", bufs=2))
        attp = ctx.enter_context(tc.tile_pool(name="attp", bufs=1))
        dramp = ctx.enter_context(tc.tile_pool(name="dramp", bufs=2, space="DRAM"))
        ppm = ctx.enter_context(tc.tile_pool(name="ppm", bufs=3, space="PSUM"))
        pps = ctx.enter_context(tc.tile_pool(name="pps", bufs=2, space="PSUM"))

        # ---- one-time constant loads (emitted first on the Pool queue) ----
        idn_sb = consts.tile([128, 128], F16, tag="idn")
        nc.sync.dma_start(idn_sb[:], idn[:])
        wdr_sb = consts.tile([128, NPC * 2 * H], F8, tag="wdr")
        nc.sync.dma_start(wdr_sb[:], wdr[:])
        fb_sb = consts.tile([SLO, SHI + BPC + BPC * SHI + 1], F32, tag="fblob")
        nc.sync.dma_start(fb_sb[:], fblob[:])
        r1lhs_sb = consts.tile([2, BPC * S], BF, tag="r1lhs")
        nc.sync.dma_start(r1lhs_sb[:], r1lhs[:])
        r1rhs_sb = consts.tile([2, BPC * H], BF, tag="r1rhs")
        nc.sync.dma_start(r1rhs_sb[:], r1rhs[:])
        vbc_sb = consts.tile([128, H + 1], BF, tag="vbc")
        nc.sync.dma_start(vbc_sb[:], vbc[:])
        brow_sb = consts.tile([1, 128], F32, tag="brow")
        nc.sync.dma_start(brow_sb[:], brow[:])

        iota_sb = fb_sb[:, 0:SHI]
        lens_sb = fb_sb[:, SHI:SHI + BPC]
        covt_sb = fb_sb[:, SHI + BPC:SHI + BPC + BPC * SHI]
        ones_cf_sb = fb_sb[:, SHI + BPC + BPC * SHI:]      # [128,1] f32 ones
        ones_c_sb = vbc_sb[:, H:H + 1]                     # [128,1] bf16 ones
        ones_r_sb = brow_sb                                # [1,128] f32 ones

        def wdr_ap(pc):  # [128, 2, H] fp8 moving pair weights
            return wdr_sb[:, pc * 2 * H:(pc + 1) * 2 * H].rearrange(
                "p (t k) -> p t k", t=2)

        # ---- per-batch cast: fp32 DRAM -> fp8 DRAM scratch [s,h], bh-halves ----
        def load_batch(b):
            e8d = dramp.tile([S, H], F8, tag="e8d")
            for bh in range(-(-njts[b] // 8)):
                rlo, rhi = bh * 1024, min(njts[b], bh * 8 + 8) * 128
                nc.gpsimd.dma_start(e8d[rlo:rhi, :], enc_f32[b, rlo:rhi, :])
            return e8d

        # xbar DMA-transposes: fp8 DRAM scratch viewed as uint16 h-pairs,
        # one transpose per (bh, pc) lands directly in the pair-interleaved
        # SBUF et2 layout that DoubleRowSwInterleave consumes (no PSUM
        # bounce, no engine copies). Returned as closures so the caller can
        # spread them through the previous batch's main loop.
        def make_rounds(e8d, b):
            njt = njts[b]
            e8u = e8d[:].bitcast(mybir.dt.uint16)    # [S, 256] pair view
            et2 = [etp.tile([128, 2 * S], F8, tag=f"et2_{pc}", name=f"et2_{pc}")
                   for pc in range(NPC)]
            nbh = -(-njt // 8)
            rounds = []
            for bh in range(nbh):
                for pc in range(NPC):
                    def emit(bh=bh, pc=pc):
                        jlo, jhi = bh * 8, min(njt, bh * 8 + 8)
                        nj = jhi - jlo
                        dst = et2[pc][:, bh * S:bh * S + nj * 256]
                        nc.sync.dma_start(
                            dst.bitcast(mybir.dt.uint16),
                            e8u[jlo * 128:jhi * 128, pc * 128:(pc + 1) * 128],
                            transpose=True)
                    rounds.append(emit)
            return et2, rounds

        # ---- masked softmax tail, emitted in 4 chunks so the serial chain
        # interleaves with the next batch's main loop on the in-order queues ----
        tail_state = {}

        def tail_chunk(b, att_t, phase):
            st = tail_state.setdefault(b, {})
            if phase == 0:
                st["expt"] = small.tile([SLO, SHI], F32, tag="expt", name="expt")
                nc.scalar.activation(st["expt"][:], att_t[:], AF.Exp)
                st["mexp"] = small.tile([SLO, SHI], F32, tag="mexp", name="mexp")
                nc.vector.scalar_tensor_tensor(
                    out=st["mexp"][:], in0=iota_sb, scalar=lens_sb[:, b:b + 1],
                    in1=st["expt"][:], op0=OP.is_lt, op1=OP.mult,
                )
                st["pst"] = pps.tile([128, 32], F32, tag="smax", name="smax")
                nc.tensor.matmul(st["pst"][0:1, 0:SHI], ones_cf_sb,
                                 st["mexp"][:],
                                 start=True, stop=True, skip_group_check=True)
            elif phase == 1:
                ssum = small.tile([1, 1], F32, tag="ssum")
                nc.vector.reduce_sum(ssum[:], st["pst"][0:1, 0:SHI],
                                     axis=mybir.AxisListType.X)
                sinv = small.tile([1, 1], F32, tag="sinv")
                nc.vector.reciprocal(sinv[:], ssum[:])
                nc.tensor.matmul(st["pst"][:, 16:17], ones_r_sb, sinv[:],
                                 start=True, stop=True, skip_group_check=True)
            elif phase == 2:
                st["wts"] = small.tile([SLO, SHI], F32, tag="wts", name="wts")
                nc.vector.tensor_scalar(st["wts"][:], st["mexp"][:],
                                        st["pst"][:, 16:17], None, OP.mult)
                nc.sync.dma_start(att_out[b], st["wts"][:])
            else:
                ncov = small.tile([SLO, SHI], F32, tag="ncov")
                nc.vector.scalar_tensor_tensor(
                    out=ncov[:], in0=st["mexp"][:], scalar=st["pst"][:, 16:17],
                    in1=covt_sb[:, b * SHI:(b + 1) * SHI],
                    op0=OP.mult, op1=OP.add,
                )
                nc.sync.dma_start(cov_out[b], ncov[:])
                del tail_state[b]

        def mains_pair(b, jp, att_t, et2):
            ps = ppm.tile([128, 2 * H], F32, tag="x")
            for g in range(2):
                j = 2 * jp + g
                psg = ps[:, g * H:(g + 1) * H]
                nc.tensor.matmul(
                    psg,
                    r1lhs_sb[:, b * S + j * 128: b * S + (j + 1) * 128],
                    r1rhs_sb[:, b * H:(b + 1) * H],
                    start=True, stop=False, skip_group_check=True,
                )
                for pc in range(NPC):
                    nc.tensor.matmul(
                        psg,
                        et2[pc][:, j * 256:(j + 1) * 256],
                        wdr_ap(pc),
                        start=False, stop=(pc == NPC - 1),
                        perf_mode=PM.DoubleRowSwInterleave,
                        skip_group_check=True,
                    )
            t_t = tpool.tile([128, 2 * H], BF, tag="t")
            nc.scalar.activation(t_t[:], ps[:], AF.Tanh, scale=1.0 / WSCALE)
            for g in range(2):
                j = 2 * jp + g
                scr = spool.tile([128, H], BF, tag=f"scr{g}")
                nc.vector.scalar_tensor_tensor(
                    out=scr[:], in0=t_t[:, g * H:(g + 1) * H], scalar=1.0,
                    in1=vbc_sb[:, 0:H],
                    op0=OP.mult, op1=OP.mult,
                    accum_out=att_t[:, j:j + 1],
                )

        # ---- stage schedule: merge latency-bound small batches so their
        # independent pipeline chains overlap ----
        stages = []
        b = 0
        while b < BPC:
            stages.append([b])
            b += 1

        pre, et2s, att_ts = {}, {}, {}
        for b in stages[0]:
            pre[b] = load_batch(b)
            et2s[b], r0 = make_rounds(pre.pop(b), b)
            for r in r0:
                r()

        prev_tails = []    # (b, att_t) awaiting tail emission
        for si, stage in enumerate(stages):
            nxt = stages[si + 1] if si + 1 < len(stages) else []
            rounds_next = []
            for nb in nxt:
                pre[nb] = load_batch(nb)
            for nb in nxt:
                et2s[nb], rs = make_rounds(pre.pop(nb), nb)
                rounds_next.extend(rs)

            # round-robin the stage's pair groups
            seq = []
            for b in stage:
                att_ts[b] = attp.tile([SLO, SHI], F32, tag=f"att{b}",
                                      name=f"att{b}")
                if njts[b] < SHI:
                    nc.gpsimd.memset(att_ts[b][:, njts[b]:], 0.0)
            for jp in range(max(njts[b] // 2 for b in stage)):
                for b in stage:
                    if jp < njts[b] // 2:
                        seq.append((b, jp))

            nchunks = 4 * len(prev_tails)
            chunks_done = 0
            npos = len(seq)
            for pos, (b, jp) in enumerate(seq):
                mains_pair(b, jp, att_ts[b], et2s[b])
                # pending softmax tails from the previous stage
                target = (pos + 1) * nchunks // npos
                while chunks_done < min(target, nchunks):
                    tb, tatt = prev_tails[chunks_done // 4]
                    tail_chunk(tb, tatt, chunks_done % 4)
                    chunks_done += 1
                # next stage's transpose rounds, late in this stage's loop
                k = pos - max(1, npos - len(rounds_next))
                if 0 <= k < len(rounds_next):
                    rounds_next[k]()
                    rounds_next[k] = None
            while chunks_done < nchunks:
                tb, tatt = prev_tails[chunks_done // 4]
                tail_chunk(tb, tatt, chunks_done % 4)
                chunks_done += 1
            for r in rounds_next:
                if r is not None:
                    r()

            prev_tails = [(b, att_ts[b]) for b in stage]

        for tb, tatt in prev_tails:
            for phase in range(4):
                tail_chunk(tb, tatt, phase)

    nc.compile()
    return nc


def _get_nc(njts=None):
    if njts is None:
        return _CACHE["last"]
    if njts not in _CACHE:
        _CACHE[njts] = _build_nc(njts)
    _CACHE["last"] = _CACHE[njts]
    return _CACHE[njts]


def _plan(text_lengths):
    """Sort batches by length, snake-assign to cores so every core sees the
    same per-slot live-tile count (SPMD), shortest slot first."""
    lens_i = np.clip(np.asarray(text_lengths).astype(np.int64), 1, S)
    order = np.argsort(-lens_i, kind="stable")
    groups = [order[8 * g:8 * g + 8] for g in range(BPC)]   # g=0 longest
    groups = groups[::-1]                                   # ascending njt
    njts = tuple(int(2 * -(-int(lens_i[g].max()) // 256)) for g in groups)
    # bidx[core][slot] = original batch index
    bidx = [[int(groups[s][c]) for s in range(BPC)] for c in range(NCORES)]
    return njts, bidx


def _prep_in_maps(dec_input, enc_output, text_lengths, coverage_vector, W, b,
                  v_w, bidx):
    enc = np.ascontiguousarray(np.asarray(enc_output, dtype=np.float32))
    dec = np.asarray(dec_input, dtype=np.float32).reshape(B, E)
    cov = np.asarray(coverage_vector, dtype=np.float32)
    W = np.asarray(W, dtype=np.float32)
    b = np.asarray(b, dtype=np.float32)
    v_w = np.asarray(v_w, dtype=np.float32)
    lens_f = np.asarray(text_lengths).astype(np.float32)

    wenc16 = (W[:H] * WSCALE).astype(F8E4)      # [h, k] fp8, x16
    wcovsum = W[H + E:].sum(axis=0, dtype=np.float32)
    dec_proj = dec @ W[H:H + E]                 # (B, H) fp32 on host
    vbc = np.empty((128, H + 1), BF16)
    vbc[:, :H] = v_w.astype(BF16)[None, :]
    vbc[:, H] = BF16(1.0)
    # SwInterleave reverses stationary columns: partition p <-> s = 128j+127-p
    iota = ((127.0 - np.arange(SLO, dtype=np.float32))[:, None]
            + 128.0 * np.arange(SHI, dtype=np.float32)[None, :])
    brow = np.ones((1, 128), np.float32)
    # cov in [p, j] layout with the s flip inside each 128-block
    cov_pj = cov.reshape(B, SHI, SLO)[:, :, ::-1].transpose(0, 2, 1)  # [B,128,SHI]

    wdr = np.zeros((128, NPC * 2 * H), F8E4)
    hh = np.arange(128)
    for pc in range(NPC):
        for t in range(2):
            rows = wenc16[pc * 256 + 2 * hh + t]            # [128, H]
            wdr[:, (pc * 2 + t) * H:(pc * 2 + t + 1) * H] = rows
    idn = np.eye(128, dtype=np.float16)

    in_maps = []
    for core in range(NCORES):
        sl = bidx[core]

        fblob = np.empty((SLO, SHI + BPC + BPC * SHI + 1), np.float32)
        fblob[:, 0:SHI] = iota
        fblob[:, SHI:SHI + BPC] = lens_f[sl][None, :]
        fblob[:, SHI + BPC:SHI + BPC + BPC * SHI] = \
            cov_pj[sl].transpose(1, 0, 2).reshape(SLO, BPC * SHI)
        fblob[:, -1] = 1.0

        r1l = np.empty((2, BPC * S), BF16)
        r1l[0] = BF16(1.0)
        # r1 columns map straight to out partitions: use the flipped layout
        r1l[1] = (cov_pj[sl].astype(BF16).transpose(0, 2, 1).reshape(-1))

        r1r = np.empty((2, BPC * H), np.float32)
        r1r[0] = (WSCALE * (dec_proj[sl] + b[None, :])).reshape(-1)
        r1r[1] = np.broadcast_to(WSCALE * wcovsum, (BPC, H)).reshape(-1)

        in_maps.append({
            "enc_f32": enc[sl],
            "wdr": wdr,
            "idn": idn,
            "fblob": fblob,
            "r1lhs": r1l,
            "r1rhs": r1r.astype(BF16),
            "vbc": vbc,
            "brow": brow,
        })
    return in_maps


def kernel(dec_input, enc_output, text_lengths, coverage_vector, W, b, v_w, v_b):
    from concourse.bass_utils import run_bass_kernel_spmd

    njts, bidx = _plan(text_lengths)
    nc = _get_nc(njts)
    in_maps = _prep_in_maps(dec_input, enc_output, text_lengths,
                            coverage_vector, W, b, v_w, bidx)
    res = run_bass_kernel_spmd(nc, in_maps, core_ids=list(range(NCORES)))

    att = np.empty((B, S), np.float32)
    ncov = np.empty((B, S), np.float32)
    for core in range(NCORES):
        r = res.results[core]
        # undo the per-128-block s flip: out partition p is s = 128j + 127 - p
        att[bidx[core]] = r["att_out"][:, ::-1, :].transpose(0, 2, 1).reshape(BPC, S)
        ncov[bidx[core]] = r["cov_out"][:, ::-1, :].transpose(0, 2, 1).reshape(BPC, S)
    return att, ncov
